# revision 30
# baseline (speedup 1.0000x reference)
"""FP8-quantized dense MLP (scaled matmul) on 8 Trainium2 NeuronCores.

Reference computation:
    x  [8, 2048, 4096] f32, weight [4096, 4096] f32
    sx = 448 / amax(|x|); sw = 448 / amax(|w|)
    out = (q8(x*sx) @ q8(w*sw)) * (1/sx) * (1/sw)     (q8 = OCP e4m3fn RNE)

Sharding: 4 M-shards x 2 N-shards over 8 cores (core c -> rows
[c//2*4096, +4096), cols [c%2*2048, +2048)).  Scales + fp8 quantization run
on host (O(MK+KN) elementwise prep); the O(MKN) matmul runs on device.

TRN2's FP8_EXP4 has max +-240 (OCP e4m3fn has +-448), so OCP-quantized values
256..448 would be NaN/Inf on device.  We therefore quantize to the OCP grid
*halved* (exact in fp8 for all but deep-subnormal values) by scaling with
sx/2 and clipping to +-224, and compensate with a *4 factor folded into the
output scale.  The device matmul (fp8 products, f32 accumulate) is then
bit-equivalent to the reference modulo f32 summation order.

Device kernel per core: out[4096, 2048] = xT.T @ w in fp8 DoubleRow mode
(K-tiles of 256), weight slab resident in SBUF, x streamed per 128-row
m-tile (x0's DMA issued ahead of the weight stream, x1's a little way into
it, so the PE starts while the weight streams in; dummy warm-up matmuls
absorb the HAM clock ramp meanwhile), 4 PSUM banks per m-tile (ps0-2 double
buffered, ps3 single + 1 warm-up bank), scaled PSUM eviction alternating
scalar/vector engines, f32 results DMA'd out.
"""

import numpy as np
import ml_dtypes

FP8_MAX = 448.0
B, S, K, N = 8, 2048, 4096, 4096
NCORES = 8
MSHARDS = 4
NSHARDS = 2
M_CORE = B * S // MSHARDS   # 4096 rows per core
N_CORE = N // NSHARDS       # 2048 cols per core
P = 128
KS = K // P      # 32 k-subtiles of 128 (partition dim)
K2 = K // 256    # 16 DoubleRow k-tiles of 256
MT = M_CORE // P  # 32 m-tiles per core
NFREE = 512      # matmul free dim == one PSUM bank of f32
NT = N_CORE // NFREE  # 4 PSUM banks per m-tile

_E4M3 = ml_dtypes.float8_e4m3  # TRN semantics: max +-240

_nc_cache = None


def _build_nc():
    from concourse import bacc, tile, mybir

    nc = bacc.Bacc("TRN2", debug=False)
    xt_d = nc.dram_tensor("xt", [MT, P, KS, P], mybir.dt.float8e4, kind="ExternalInput")
    wt_d = nc.dram_tensor(
        "wt", [K2, P, 2, N_CORE], mybir.dt.float8e4, kind="ExternalInput"
    )
    sc_d = nc.dram_tensor("sc", [P, 1], mybir.dt.float32, kind="ExternalInput")
    out_d = nc.dram_tensor("out", [M_CORE, N_CORE], mybir.dt.float32, kind="ExternalOutput")

    with tile.TileContext(nc) as tc:
        with (
            tc.tile_pool(name="wp", bufs=1) as wp,
            tc.tile_pool(name="xp", bufs=4) as xp,
            tc.tile_pool(name="op", bufs=4) as op,
            tc.tile_pool(name="cp", bufs=1) as cp,
            tc.tile_pool(name="pp", bufs=2, space="PSUM") as pp,
        ):
            # PE warm-up: the HAM clock gate keeps the PE at 1.2 GHz until it
            # has been busy ~3.4us.  Dummy matmuls on zeroed scratch during
            # the input-DMA prologue absorb the ramp so the real stream runs
            # at 2.4 GHz from its first instruction.  The warm-up PSUM tile
            # gets its OWN bank (tag ps3 drops to one slot below) — sharing a
            # bank with a live accumulator tag crashes the device
            # (PSUM_COLLISION).
            wa = cp.tile([P, 2, P], mybir.dt.float8e4, tag="wa")
            wb = cp.tile([P, 2, 2 * P], mybir.dt.float8e4, tag="wb")
            nc.vector.memset(wa[:], 0)
            nc.vector.memset(wb[:], 0)
            psw = pp.tile([P, 2 * P], mybir.dt.float32, tag="psw", bufs=1, name="psw")
            for _ in range(14):
                nc.tensor.matmul(
                    psw[:],
                    wa[:],
                    wb[:],
                    start=True,
                    stop=True,
                    perf_mode=mybir.MatmulPerfMode.DoubleRow,
                )

            sc_sb = cp.tile([P, 1], mybir.dt.float32, tag="sc")

            # Input DMAs share one queue, so program order == arrival order:
            # x0 and w0 first (the first matmul needs both — every trigger
            # ahead of them delays the real stream by ~0.7us of trigger time
            # plus transfer), then the tiny sc scale (first needed by the
            # first eviction ~25us in), x1 a little way into the weight
            # stream (needed only ~15us later).  All DMA triggers stay on
            # nc.sync: routing any through gpsimd measured a 95us regression
            # (slow software-DGE completion backpressures the eviction
            # pipeline and ultimately the PE).
            x_tiles = {}
            for m in range(2):
                x_tiles[m] = xp.tile(
                    [P, KS, P], mybir.dt.float8e4, tag="x", name=f"x{m}"
                )
            nc.sync.dma_start(x_tiles[0][:], xt_d[0])

            w_sb = []
            for k2 in range(K2):
                w_t = wp.tile([P, 2, N_CORE], mybir.dt.float8e4, tag=f"w{k2}")
                nc.sync.dma_start(w_t[:], wt_d[k2])
                w_sb.append(w_t)
                if k2 == 0:
                    nc.sync.dma_start(sc_sb[:], sc_d[:])
                if k2 == 2:
                    nc.sync.dma_start(x_tiles[1][:], xt_d[1])

            def alloc_ps(m):
                # ps3 single-buffered: its double-buffer slot is the warm-up
                # bank (PSUM holds exactly 8 banks; evictions are ~20x faster
                # than an m-tile, so one tag without WAR slack costs nothing)
                return [
                    pp.tile(
                        [P, NFREE],
                        mybir.dt.float32,
                        tag=f"ps{n}",
                        name=f"ps{m}_{n}",
                        bufs=1 if n == NT - 1 else 2,
                    )
                    for n in range(NT)
                ]

            def mm_k2(x_t, ps, k2):
                lhs = x_t[:, 2 * k2 : 2 * k2 + 2, :]
                for n in range(NT):
                    nc.tensor.matmul(
                        ps[n][:],
                        lhs,
                        w_sb[k2][:, :, n * NFREE : (n + 1) * NFREE],
                        start=(k2 == 0),
                        stop=(k2 == K2 - 1),
                        perf_mode=mybir.MatmulPerfMode.DoubleRow,
                    )

            def evict(m, ps):
                for n in range(NT):
                    o_t = op.tile([P, NFREE], mybir.dt.float32, tag="o", name=f"o{m}_{n}")
                    if n % 2 == 0:
                        nc.scalar.activation(
                            o_t[:],
                            ps[n][:],
                            mybir.ActivationFunctionType.Copy,
                            scale=sc_sb[:],
                        )
                    else:
                        nc.vector.tensor_scalar_mul(o_t[:], ps[n][:], sc_sb[:])
                    nc.sync.dma_start(
                        out_d[m * P : (m + 1) * P, n * NFREE : (n + 1) * NFREE],
                        o_t[:],
                    )

            for m in range(MT):
                if m in x_tiles:
                    x_t = x_tiles.pop(m)
                else:
                    x_t = xp.tile([P, KS, P], mybir.dt.float8e4, tag="x", name=f"x{m}")
                    nc.sync.dma_start(x_t[:], xt_d[m])
                ps = alloc_ps(m)
                for k2 in range(K2):
                    mm_k2(x_t, ps, k2)
                evict(m, ps)

    nc.finalize()
    return nc


def _get_nc():
    global _nc_cache
    if _nc_cache is None:
        _nc_cache = _build_nc()
    return _nc_cache


def _amax(a):
    # max(|a|) without a full |a| temp; exact (max/min are exact in f32)
    return np.float32(max(np.float32(a.max()), -np.float32(a.min())))


def _prep(x, weight):
    """Host prep: scales, halved OCP-grid fp8 quantization, tiled layouts."""
    x = np.asarray(x, dtype=np.float32)
    weight = np.asarray(weight, dtype=np.float32)

    sx = np.float32(FP8_MAX) / np.maximum(_amax(x), np.float32(1e-12))
    sw = np.float32(FP8_MAX) / np.maximum(_amax(weight), np.float32(1e-12))
    clip = np.float32(FP8_MAX / 2.0)  # 224

    # weight: [K, N] -> per N-shard [K2, P, 2, N_CORE]:
    #   wt[k2, ki, i, n] = wq[k2*256 + i*128 + ki, nh*N_CORE + n]
    wbuf = weight * (sw * np.float32(0.5))
    np.clip(wbuf, -clip, clip, out=wbuf)
    wq = wbuf.astype(_E4M3)
    wts = [
        np.ascontiguousarray(
            wq[:, nh * N_CORE : (nh + 1) * N_CORE]
            .reshape(K2, 2, P, N_CORE)
            .transpose(0, 2, 1, 3)
        )
        for nh in range(NSHARDS)
    ]

    # x per M-shard ms: rows [ms*4096, +4096) -> [MT, P, KS, P] with
    # xt[m, ki, ks, j] = xq[m*128+j, ks*128+ki]
    x2 = x.reshape(B * S, K)
    xts = []
    for ms in range(MSHARDS):
        xbuf = x2[ms * M_CORE : (ms + 1) * M_CORE] * (sx * np.float32(0.5))
        np.clip(xbuf, -clip, clip, out=xbuf)
        xq = xbuf.astype(_E4M3)
        xts.append(np.ascontiguousarray(xq.reshape(MT, P, KS, P).transpose(0, 3, 2, 1)))

    # output scale: psum = ref_matmul / 4  ->  multiply by 4 * (1/sx) * (1/sw)
    c = np.float32(4.0) * (np.float32(1.0) / sx) * (np.float32(1.0) / sw)
    sc = np.full((P, 1), c, dtype=np.float32)
    return xts, wts, sc


def _run(x, weight, trace=False, tmpdir=None):
    from concourse.bass_utils import run_bass_kernel_spmd

    nc = _get_nc()
    xts, wts, sc = _prep(x, weight)
    in_maps = [
        {"xt": xts[c // NSHARDS], "wt": wts[c % NSHARDS], "sc": sc}
        for c in range(NCORES)
    ]
    res = run_bass_kernel_spmd(
        nc, in_maps, list(range(NCORES)), trace=trace, tmpdir=tmpdir
    )
    out = np.empty((B * S, N), dtype=np.float32)
    for c in range(NCORES):
        ms, nh = c // NSHARDS, c % NSHARDS
        out[ms * M_CORE : (ms + 1) * M_CORE, nh * N_CORE : (nh + 1) * N_CORE] = (
            res.results[c]["out"]
        )
    return out.reshape(B, S, N), res


def kernel(x, weight):
    out, _ = _run(x, weight, trace=False)
    return out


def run_traced(x, weight, tmpdir=None):
    """For test harnesses: returns (out, exec_time_ns)."""
    out, res = _run(x, weight, trace=True, tmpdir=tmpdir)
    return out, res.exec_time_ns
